# revision 3
# baseline (speedup 1.0000x reference)
"""Multi-head self-attention (causal, 16 heads, d_model=1024, S=4096) on 8 TRN2 cores.

Strategy (tensor-parallel over heads + AllToAll):
  - Each core owns 2 heads. It computes Q^T/K^T/V projections for its heads
    over the full sequence (fp32r matmuls), then causal flash-style attention:
      scores S^T[k,q] = K_tile^T @ Q  (per 128-k-tile x 256-q-tile, PSUM)
      exp on ScalarE (scale=1/8 folded in), causal mask on diag tiles (DVE mult)
      O^T[dk,q] accumulated in PSUM via lhsT=V_aug (V with a ones column ->
      row 64 of O^T is the softmax denominator d[q])
      normalize: d broadcast via K=1 matmul, reciprocal_approx_fast, multiply
  - AllToAll exchanges head-slices for sequence-slices: core c ends up with
    A^T[1024, 512] for its own 512-row window (all static addressing).
  - Output projection per core: out[s,dout] = A^T-tiles.T @ Wo^T, rows
    [512c, 512c+512) of the final output. Host concatenates windows.

All matmuls run as float32r (TF32-like, ~1e-3 rel err, 4x faster than fp32).
"""

import numpy as np

import concourse.bass as bass
import concourse.bacc as bacc
import concourse.tile as tile
import concourse.mybir as mybir
from concourse import bass_utils
from concourse.masks import make_identity

D = 1024          # model dim
H = 16            # heads
DK = 64           # head dim
NCORES = 8
HPC = H // NCORES  # heads per core = 2
HD = HPC * DK      # local head dims = 128

F32 = mybir.dt.float32
F32R = mybir.dt.float32r
EXP = mybir.ActivationFunctionType.Exp


def build(S: int = 4096) -> bacc.Bacc:
    assert S % 512 == 0
    WIN = S // NCORES          # own output window rows
    NQT = S // 256             # 256-wide q tiles
    NKT = S // 128             # 128-row k tiles
    NST = S // 512             # 512-wide projection s-tiles
    NSC = WIN // 128           # 128-row output s-chunks

    nc = bacc.Bacc("TRN2", target_bir_lowering=False, debug=False,
                   num_devices=NCORES)

    xT_d = nc.dram_tensor("xT", [D, S], F32R, kind="ExternalInput")
    wqT_d = nc.dram_tensor("wqT", [D, HD], F32R, kind="ExternalInput")
    wkT_d = nc.dram_tensor("wkT", [D, HD], F32R, kind="ExternalInput")
    wvT_d = nc.dram_tensor("wvT", [D, HD], F32R, kind="ExternalInput")
    woT_d = nc.dram_tensor("woT", [D, D], F32R, kind="ExternalInput")
    mask_d = nc.dram_tensor("mask", [128, 384], F32R, kind="ExternalInput")
    out_d = nc.dram_tensor("out", [WIN, D], F32, kind="ExternalOutput")

    rg = [list(range(NCORES))]

    with tile.TileContext(nc) as tc:
        with (
            tc.tile_pool(name="persist", bufs=1) as persist,
            tc.tile_pool(name="dram", bufs=1, space="DRAM") as dram,
        ):
            # ---- persistent SBUF ----
            wq_sb = persist.tile([128, 8, HD], F32R)
            wk_sb = persist.tile([128, 8, HD], F32R)
            wv_sb = persist.tile([128, 8, HD], F32R)
            wo_sb = persist.tile([128, 8, D], F32R)
            mask_sb = persist.tile([128, 384], F32R)
            ident = persist.tile([128, 128], F32)
            ones64 = persist.tile([128, 64], F32R)
            qT_sb = persist.tile([128, S], F32R)      # 2 heads stacked [dq, s]
            kT_sb = persist.tile([128, S], F32R)
            vaug = [persist.tile([128, NKT, DK + 1], F32R, name=f"vaug{h}")
                    for h in range(HPC)]

            nc.sync.dma_start(wq_sb[:], wqT_d.ap().rearrange("(dt p) m -> p dt m", p=128))
            nc.sync.dma_start(wk_sb[:], wkT_d.ap().rearrange("(dt p) m -> p dt m", p=128))
            nc.sync.dma_start(wv_sb[:], wvT_d.ap().rearrange("(dt p) m -> p dt m", p=128))
            nc.sync.dma_start(wo_sb[:], woT_d.ap().rearrange("(hb p) o -> p hb o", p=128))
            nc.sync.dma_start(mask_sb[:], mask_d.ap())
            make_identity(nc, ident[:])
            ones_f32 = persist.tile([128, 64], F32)
            nc.vector.memset(ones_f32[:], 1.0)
            nc.vector.tensor_copy(ones64[:], ones_f32[:])
            for h in range(HPC):
                nc.vector.tensor_copy(
                    vaug[h][:, :, DK:DK + 1],
                    ones_f32[:, 0:1].to_broadcast([128, NKT, 1]))

            a2a_in = dram.tile([NCORES, HD, WIN], F32R)
            a2a_out = dram.tile([NCORES, HD, WIN], F32R)

            xT_ap = xT_d.ap().rearrange("(dt p) s -> p dt s", p=128)

            # ================= Phase 1: QKV projections =================
            with (
                tc.tile_pool(name="xs_pool", bufs=3) as xs_pool,
                tc.tile_pool(name="vt_pool", bufs=2) as vt_pool,
                tc.tile_pool(name="ps1", bufs=3, space="PSUM") as ps1,
                tc.tile_pool(name="ps1t", bufs=2, space="PSUM") as ps1t,
            ):
                for st in range(NST):
                    xs = xs_pool.tile([128, 8, 512], F32R)
                    nc.sync.dma_start(xs[:], xT_ap[:, :, st * 512:(st + 1) * 512])
                    ssl = slice(st * 512, (st + 1) * 512)
                    for w_sb, dstT in ((wq_sb, qT_sb), (wk_sb, kT_sb)):
                        ps = ps1.tile([128, 512], F32)
                        for dt in range(8):
                            nc.tensor.matmul(ps[:], w_sb[:, dt, :], xs[:, dt, :],
                                             start=(dt == 0), stop=(dt == 7))
                        nc.vector.tensor_copy(dstT[:, ssl], ps[:])
                    # V^T then transpose into V_aug (per head)
                    psv = ps1.tile([128, 512], F32)
                    for dt in range(8):
                        nc.tensor.matmul(psv[:], wv_sb[:, dt, :], xs[:, dt, :],
                                         start=(dt == 0), stop=(dt == 7))
                    vT = vt_pool.tile([128, 512], F32)
                    nc.vector.tensor_copy(vT[:], psv[:])
                    for j in range(4):
                        kt = st * 4 + j
                        pst = ps1t.tile([128, 128], F32)
                        nc.tensor.transpose(pst[:], vT[:, j * 128:(j + 1) * 128], ident[:])
                        for h in range(HPC):
                            nc.vector.tensor_copy(vaug[h][:, kt, 0:DK],
                                                  pst[:, h * DK:(h + 1) * DK])

            # ================= Phase 2: causal attention =================
            with (
                tc.tile_pool(name="exp_pool", bufs=6) as exp_pool,
                tc.tile_pool(name="o_pool", bufs=3) as o_pool,
                tc.tile_pool(name="rc_pool", bufs=3) as rc_pool,
                tc.tile_pool(name="an_pool", bufs=3) as an_pool,
                tc.tile_pool(name="ps_s", bufs=3, space="PSUM") as ps_s_pool,
                tc.tile_pool(name="ps_av", bufs=2, space="PSUM") as ps_av_pool,
                tc.tile_pool(name="ps_db", bufs=2, space="PSUM") as ps_db_pool,
            ):
                for t in range(NQT):
                    qsl = slice(t * 256, (t + 1) * 256)
                    nkt = 2 * t + 2
                    for h in range(HPC):
                        hsl = slice(h * DK, (h + 1) * DK)
                        ps_av = ps_av_pool.tile([DK + 1, 256], F32)
                        for kt in range(nkt):
                            ps_s = ps_s_pool.tile([128, 256], F32)
                            nc.tensor.matmul(ps_s[:],
                                             kT_sb[hsl, kt * 128:(kt + 1) * 128],
                                             qT_sb[hsl, qsl],
                                             start=True, stop=True)
                            ex = exp_pool.tile([128, 256], F32R)
                            nc.scalar.activation(ex[:], ps_s[:], EXP, scale=0.125)
                            if kt == 2 * t:
                                nc.vector.tensor_mul(ex[:], ex[:], mask_sb[:, 128:384])
                            elif kt == 2 * t + 1:
                                nc.vector.tensor_mul(ex[:], ex[:], mask_sb[:, 0:256])
                            nc.tensor.matmul(ps_av[:], vaug[h][:, kt, :], ex[:],
                                             start=(kt == 0), stop=(kt == nkt - 1))
                        o_sb = o_pool.tile([DK + 1, 256], F32R)
                        nc.vector.tensor_copy(o_sb[:], ps_av[:])
                        ps_db = ps_db_pool.tile([DK, 256], F32)
                        nc.tensor.matmul(ps_db[:], ones64[DK:DK + 1, :],
                                         o_sb[DK:DK + 1, :], start=True, stop=True)
                        rc = rc_pool.tile([DK, 256], F32)
                        nc.vector.reciprocal_approx_fast(rc[:], ps_db[:])
                        an = an_pool.tile([DK, 256], F32R)
                        nc.vector.tensor_mul(an[:], o_sb[0:DK, :], rc[:])
                        w, off = divmod(t * 256, WIN)
                        nc.sync.dma_start(
                            a2a_in[w, h * DK:(h + 1) * DK, off:off + 256], an[:])

            # ================= Phase 3: AllToAll =================
            nc.gpsimd.collective_compute(
                "AllToAll", mybir.AluOpType.bypass, replica_groups=rg,
                ins=[a2a_in.opt()], outs=[a2a_out.opt()])

            # ================= Phase 4: output projection =================
            with (
                tc.tile_pool(name="lhs_pool", bufs=16) as lhs_pool,
                tc.tile_pool(name="ob_pool", bufs=3) as ob_pool,
                tc.tile_pool(name="ps4", bufs=2, space="PSUM") as ps4,
            ):
                for c in range(NSC):
                    csl = slice(c * 128, (c + 1) * 128)
                    lhs = []
                    for hb in range(8):
                        lt = lhs_pool.tile([128, 128], F32R, tag="lhs")
                        nc.sync.dma_start(lt[:], a2a_out[hb, :, csl])
                        lhs.append(lt)
                    for do in range(2):
                        ps_o = ps4.tile([128, 512], F32)
                        for hb in range(8):
                            nc.tensor.matmul(ps_o[:], lhs[hb][:],
                                             wo_sb[:, hb, do * 512:(do + 1) * 512],
                                             start=(hb == 0), stop=(hb == 7))
                        ob = ob_pool.tile([128, 512], F32)
                        nc.vector.tensor_copy(ob[:], ps_o[:])
                        nc.sync.dma_start(
                            out_d.ap()[csl, do * 512:(do + 1) * 512], ob[:])

    nc.compile()
    return nc


def make_mask() -> np.ndarray:
    """[zeros(128) | tril-keep(128) | ones(128)] -> [128, 384] multiplicative."""
    m = np.zeros((128, 384), np.float32)
    k = np.arange(128)[:, None]
    q = np.arange(128)[None, :]
    m[:, 128:256] = (k <= q).astype(np.float32)
    m[:, 256:384] = 1.0
    return m


def make_in_maps(x, Wq, Wk, Wv, Wo, S):
    xT = np.ascontiguousarray(x.reshape(S, D).T).astype(np.float32)
    woT = np.ascontiguousarray(Wo.T).astype(np.float32)
    mask = make_mask()
    in_maps = []
    for c in range(NCORES):
        hsl = slice(c * HD, (c + 1) * HD)
        in_maps.append({
            "xT": xT,
            "wqT": np.ascontiguousarray(Wq[hsl, :].T).astype(np.float32),
            "wkT": np.ascontiguousarray(Wk[hsl, :].T).astype(np.float32),
            "wvT": np.ascontiguousarray(Wv[hsl, :].T).astype(np.float32),
            "woT": woT,
            "mask": mask,
        })
    return in_maps


_cache: dict = {}


def kernel(x: np.ndarray, Wq: np.ndarray, Wk: np.ndarray, Wv: np.ndarray,
           Wo: np.ndarray) -> np.ndarray:
    B, S, Dm = x.shape
    assert B == 1 and Dm == D
    if S not in _cache:
        _cache[S] = build(S)
    nc = _cache[S]
    in_maps = make_in_maps(x, Wq, Wk, Wv, Wo, S)
    res = bass_utils.run_bass_kernel_spmd(nc, in_maps, core_ids=list(range(NCORES)))
    out = np.concatenate([res.results[c]["out"] for c in range(NCORES)], axis=0)
    return out.reshape(1, S, D).astype(np.float32)


# revision 22
# speedup vs baseline: 331.8937x; 331.8937x over previous
"""Multi-head self-attention (causal, 16 heads, d_model=1024, S=4096) on 8 TRN2 cores.

Strategy (tensor-parallel over heads + AllToAll):
  - Each core owns 2 heads. Fused pipeline over 512-row chunks st:
    project Q^T/K^T/V for chunk st (fp32r matmuls), then causal attention for
    q-window t=st (its k-range [0, 512(st+1)) is exactly what's projected).
      scores S^T[k,q] per 128-k-tile; pairs of k-tiles share a [128,1024]
      PSUM group -> one exp on ScalarE (scale=1/8 folded); causal mask on
      diagonal tiles (GpSimd mult); O^T accumulated in PSUM via lhsT=V_aug
      (V with a ones column -> row 64 of O^T is the softmax denominator)
      normalize: d broadcast via K=1 matmul, reciprocal_approx_fast, multiply
  - AllToAll exchanges head-slices for sequence-slices: core c ends up with
    A^T[1024, 512] for its own 512-row window (all static addressing).
  - Output projection per core: out[s,dout] = A^T-tiles.T @ Wo^T, rows
    [512c, 512c+512) of the final output. Host concatenates windows.

All matmuls run as float32r (TF32-like, ~1e-3 rel err, 4x faster than fp32).
"""

import numpy as np

import concourse.bass as bass
import concourse.bacc as bacc
import concourse.tile as tile
import concourse.mybir as mybir
from concourse import bass_utils
from concourse.masks import make_identity

D = 1024          # model dim
H = 16            # heads
DK = 64           # head dim
NCORES = 8
HPC = H // NCORES  # heads per core = 2
HD = HPC * DK      # local head dims = 128

F32 = mybir.dt.float32
F32R = mybir.dt.float32r
EXP = mybir.ActivationFunctionType.Exp


def build(S: int = 4096, no_collective: bool = False,
          phases: tuple = (1, 2, 3, 4)) -> bacc.Bacc:
    assert S % 512 == 0
    WIN = S // NCORES          # own output window rows
    NST = S // 512             # 512-wide chunks (proj s-tiles == q-windows)
    NSC = WIN // 128           # 128-row output s-chunks

    nc = bacc.Bacc("TRN2", target_bir_lowering=False, debug=False,
                   num_devices=NCORES)

    xT_d = nc.dram_tensor("xT", [S // 512, 128, 8, 512], F32R, kind="ExternalInput")
    wqT_d = nc.dram_tensor("wqT", [128, 8, HD], F32R, kind="ExternalInput")
    wkT_d = nc.dram_tensor("wkT", [128, 8, HD], F32R, kind="ExternalInput")
    wvT_d = nc.dram_tensor("wvT", [128, 8, HD], F32R, kind="ExternalInput")
    woT_d = nc.dram_tensor("woT", [128, 8, D], F32R, kind="ExternalInput")
    mask_d = nc.dram_tensor("mask", [128, 896], F32R, kind="ExternalInput")
    out_d = nc.dram_tensor("out", [WIN, D], F32, kind="ExternalOutput")

    rg = [list(range(NCORES))]
    run_attn = 2 in phases

    with tile.TileContext(nc) as tc:
        with (
            tc.tile_pool(name="persist", bufs=1) as persist,
            tc.tile_pool(name="dram", bufs=1, space="DRAM") as dram,
        ):
            # ---- persistent SBUF ----
            wq_sb = persist.tile([128, 8, HD], F32R)
            wk_sb = persist.tile([128, 8, HD], F32R)
            wv_sb = persist.tile([128, 8, HD], F32R)
            wo_sb = persist.tile([128, 8, D], F32R)
            mask_sb = persist.tile([128, 896], F32R)
            ident = persist.tile([128, 128], F32)
            ones64 = persist.tile([128, 64], F32R)
            # per-chunk projection outputs (fine-grained deps)
            qs = [persist.tile([128, 512], F32R, name=f"qs{i}")
                  for i in range(NST)]
            ks = [persist.tile([128, 512], F32R, name=f"ks{i}")
                  for i in range(NST)]
            va = [persist.tile([128, 4, 2 * (DK + 1)], F32R, name=f"va{i}")
                  for i in range(NST)]

            nc.sync.dma_start(wq_sb[:, 0:1, :], wqT_d.ap()[:, 0:1, :])
            nc.sync.dma_start(wq_sb[:, 1:, :], wqT_d.ap()[:, 1:, :])
            make_identity(nc, ident[:])
            ones_f32 = persist.tile([128, 64], F32)
            nc.vector.memset(ones_f32[:], 1.0)
            nc.vector.tensor_copy(ones64[:], ones_f32[:])
            for i in range(NST):
                for h in range(HPC):
                    nc.vector.tensor_copy(
                        va[i][:, :, h * (DK + 1) + DK:h * (DK + 1) + DK + 1],
                        ones_f32[:, 0:1].to_broadcast([128, 4, 1]))

            a2a_in = dram.tile([NCORES, HD, WIN], F32R)
            a2a_out = dram.tile([NCORES, HD, WIN], F32R)


            # ============ fused projection + attention pipeline ============
            with (
                tc.tile_pool(name="xs_pool", bufs=3) as xs_pool,
                tc.tile_pool(name="vt_pool", bufs=2) as vt_pool,
                tc.tile_pool(name="exp_pool", bufs=8) as exp_pool,
                tc.tile_pool(name="o_pool", bufs=4) as o_pool,
                tc.tile_pool(name="rc_pool", bufs=4) as rc_pool,
                tc.tile_pool(name="an_pool", bufs=4) as an_pool,
                tc.tile_pool(name="ps1", bufs=2, space="PSUM") as ps1,
                tc.tile_pool(name="ps_s", bufs=2, space="PSUM") as ps_s_pool,
                tc.tile_pool(name="ps_av", bufs=2, space="PSUM") as ps_av_pool,
            ):
                xs0 = xs_pool.tile([128, 8, 512], F32R, tag="xs", name="xs0")
                nc.sync.dma_start(xs0[:, 0:1, :], xT_d.ap()[0][:, 0:1, :])
                nc.sync.dma_start(xs0[:, 1:, :], xT_d.ap()[0][:, 1:, :])
                nc.sync.dma_start(wk_sb[:], wkT_d.ap())
                nc.sync.dma_start(wv_sb[:], wvT_d.ap())
                nc.sync.dma_start(mask_sb[:], mask_d.ap())
                for st in range(NST):
                    # ---- projections for chunk st ----
                    if 1 in phases:
                        if st == 0:
                            xs = xs0
                        else:
                            xs = xs_pool.tile([128, 8, 512], F32R, tag="xs", name="xs")
                            nc.sync.dma_start(xs[:], xT_d.ap()[st])
                        for w_sb, dst in ((wq_sb, qs[st]), (wk_sb, ks[st])):
                            ps = ps1.tile([128, 512], F32, tag="ps1", name="ps_qk")
                            for dt in range(8):
                                nc.tensor.matmul(ps[:], w_sb[:, dt, :], xs[:, dt, :],
                                                 start=(dt == 0), stop=(dt == 7))
                            nc.vector.tensor_copy(dst[:], ps[:])
                        psv = ps1.tile([128, 512], F32, tag="ps1", name="ps_v")
                        for dt in range(8):
                            nc.tensor.matmul(psv[:], wv_sb[:, dt, :], xs[:, dt, :],
                                             start=(dt == 0), stop=(dt == 7))
                        vT = vt_pool.tile([128, 512], F32)
                        nc.vector.tensor_copy(vT[:], psv[:])
                        for j in range(4):
                            pst = ps1.tile([128, 512], F32, tag="ps1", name="ps_t")
                            nc.tensor.transpose(pst[:, 0:128],
                                                vT[:, j * 128:(j + 1) * 128], ident[:])
                            nc.vector.tensor_copy(
                                va[st][:, j, :].rearrange(
                                    "p (h x) -> p h x", h=2)[:, :, 0:DK],
                                pst[:, 0:128].rearrange("p (h d) -> p h d", h=2))

                    # ---- attention for q-window t = st ----
                    if not run_attn:
                        continue
                    t = st
                    ngrp = 2 * (t + 1)
                    ps_av = [ps_av_pool.tile([DK + 1, 512], F32, tag="ps_av",
                                             name=f"ps_av{_h}")
                             for _h in range(HPC)]
                    for g in range(ngrp):
                        for h in range(HPC):
                            hsl = slice(h * DK, (h + 1) * DK)
                            ps_grp = ps_s_pool.tile([128, 1024], F32, tag="ps_s",
                                                    name="ps_sg")
                            for ktl in range(2):
                                kt = 2 * g + ktl
                                kst, kidx = divmod(kt, 4)
                                nc.tensor.matmul(
                                    ps_grp[:, ktl * 512:(ktl + 1) * 512],
                                    ks[kst][hsl, kidx * 128:(kidx + 1) * 128],
                                    qs[t][hsl, :], start=True, stop=True)
                            ex = exp_pool.tile([128, 1024], F32R)
                            nc.scalar.activation(ex[:], ps_grp[:], EXP,
                                                 scale=0.125)
                            for ktl in range(2):
                                kt = 2 * g + ktl
                                kst, kidx = divmod(kt, 4)
                                dj = kt - 4 * t
                                exs = ex[:, ktl * 512:(ktl + 1) * 512]
                                if 0 <= dj <= 3:
                                    nc.vector.tensor_mul(
                                        exs, exs,
                                        mask_sb[:, (3 - dj) * 128:(3 - dj) * 128 + 512])
                                nc.tensor.matmul(
                                    ps_av[h][:],
                                    va[kst][:, kidx,
                                            h * (DK + 1):(h + 1) * (DK + 1)],
                                    exs, start=(g == 0 and ktl == 0),
                                    stop=(g == ngrp - 1 and ktl == 1))
                    for h in range(HPC):
                        o_sb = o_pool.tile([DK + 1, 512], F32R)
                        nc.vector.tensor_copy(o_sb[:], ps_av[h][:])
                        ps_db = ps_av_pool.tile([DK + 1, 512], F32, tag="ps_av",
                                                name="ps_db")
                        nc.tensor.matmul(ps_db[0:DK, :], ones64[DK:DK + 1, :],
                                         o_sb[DK:DK + 1, :], start=True, stop=True)
                        rc = rc_pool.tile([DK, 512], F32)
                        nc.vector.reciprocal_approx_fast(rc[:], ps_db[0:DK, :])
                        an = an_pool.tile([DK, 512], F32R)
                        nc.vector.tensor_mul(an[:], o_sb[0:DK, :], rc[:])
                        for half in range(2):
                            w, off = divmod(t * 512 + half * 256, WIN)
                            nc.sync.dma_start(
                                a2a_in[w, h * DK:(h + 1) * DK, off:off + 256],
                                an[:, half * 256:(half + 1) * 256])

            # ================= Phase 3: AllToAll =================
            if 3 in phases:
                if no_collective:
                    nc.sync.dma_start(a2a_out[:], a2a_in[:])
                else:
                    nc.gpsimd.collective_compute(
                        "AllToAll", mybir.AluOpType.bypass, replica_groups=rg,
                        ins=[a2a_in.opt()], outs=[a2a_out.opt()])

            # ================= Phase 4: output projection =================
            if 4 in phases:
              nc.sync.dma_start(wo_sb[:], woT_d.ap())
              with (
                tc.tile_pool(name="lhs_pool", bufs=16) as lhs_pool,
                tc.tile_pool(name="ob_pool", bufs=3) as ob_pool,
                tc.tile_pool(name="ps4", bufs=2, space="PSUM") as ps4,
              ):
                for c in range(NSC):
                    csl = slice(c * 128, (c + 1) * 128)
                    lhs = []
                    for hb in range(8):
                        lt = lhs_pool.tile([128, 128], F32R, tag="lhs", name="lhs")
                        nc.sync.dma_start(lt[:], a2a_out[hb, :, csl])
                        lhs.append(lt)
                    for do in range(2):
                        ps_o = ps4.tile([128, 512], F32)
                        for hb in range(8):
                            nc.tensor.matmul(ps_o[:], lhs[hb][:],
                                             wo_sb[:, hb, do * 512:(do + 1) * 512],
                                             start=(hb == 0), stop=(hb == 7))
                        ob = ob_pool.tile([128, 512], F32)
                        nc.vector.tensor_copy(ob[:], ps_o[:])
                        nc.sync.dma_start(
                            out_d.ap()[csl, do * 512:(do + 1) * 512], ob[:])

    nc.compile()
    return nc


def make_mask() -> np.ndarray:
    """[Z Z Z T 1 1 1] -> [128, 896] multiplicative; slice (3-dj)*128 : +512
    masks a 128-k-tile at diag offset dj within a 512-q window."""
    m = np.zeros((128, 896), np.float32)
    k = np.arange(128)[:, None]
    q = np.arange(128)[None, :]
    m[:, 384:512] = (k <= q).astype(np.float32)
    m[:, 512:] = 1.0
    return m


def _arr_w(wT):
    # [D, M] -> [128, 8, M]: partition p holds din rows {dt*128+p}
    return np.ascontiguousarray(wT.reshape(8, 128, -1).transpose(1, 0, 2))


def make_in_maps(x, Wq, Wk, Wv, Wo, S):
    # xarr[st, p, dt, s] = x[512*st + s, 128*dt + p]
    xarr = np.ascontiguousarray(
        x.reshape(S // 512, 512, 8, 128).transpose(0, 3, 2, 1)).astype(np.float32)
    woT = _arr_w(Wo.T).astype(np.float32)
    mask = make_mask()
    in_maps = []
    for c in range(NCORES):
        hsl = slice(c * HD, (c + 1) * HD)
        in_maps.append({
            "xT": xarr,
            "wqT": _arr_w(Wq[hsl, :].T).astype(np.float32),
            "wkT": _arr_w(Wk[hsl, :].T).astype(np.float32),
            "wvT": _arr_w(Wv[hsl, :].T).astype(np.float32),
            "woT": woT,
            "mask": mask,
        })
    return in_maps


_cache: dict = {}


def kernel(x: np.ndarray, Wq: np.ndarray, Wk: np.ndarray, Wv: np.ndarray,
           Wo: np.ndarray) -> np.ndarray:
    B, S, Dm = x.shape
    assert B == 1 and Dm == D
    if S not in _cache:
        _cache[S] = build(S)
    nc = _cache[S]
    in_maps = make_in_maps(x, Wq, Wk, Wv, Wo, S)
    res = bass_utils.run_bass_kernel_spmd(nc, in_maps, core_ids=list(range(NCORES)))
    out = np.concatenate([res.results[c]["out"] for c in range(NCORES)], axis=0)
    return out.reshape(1, S, D).astype(np.float32)


# revision 24
# speedup vs baseline: 339.6284x; 1.0233x over previous
"""Multi-head self-attention (causal, 16 heads, d_model=1024, S=4096) on 8 TRN2 cores.

Strategy (tensor-parallel over heads + AllToAll):
  - Each core owns 2 heads. Fused pipeline over 512-row chunks st:
    project Q^T/K^T/V for chunk st (fp32r matmuls), then causal attention for
    q-window t=st (its k-range [0, 512(st+1)) is exactly what's projected).
      scores S^T[k,q] per 128-k-tile; pairs of k-tiles share a [128,1024]
      PSUM group -> one exp on ScalarE (scale=1/8 folded); causal mask on
      diagonal tiles (GpSimd mult); O^T accumulated in PSUM via lhsT=V_aug
      (V with a ones column -> row 64 of O^T is the softmax denominator)
      normalize: d broadcast via K=1 matmul, reciprocal_approx_fast, multiply
  - AllToAll exchanges head-slices for sequence-slices: core c ends up with
    A^T[1024, 512] for its own 512-row window (all static addressing).
  - Output projection per core: out[s,dout] = A^T-tiles.T @ Wo^T, rows
    [512c, 512c+512) of the final output. Host concatenates windows.

All matmuls run as float32r (TF32-like, ~1e-3 rel err, 4x faster than fp32).
"""

import numpy as np

import concourse.bass as bass
import concourse.bacc as bacc
import concourse.tile as tile
import concourse.mybir as mybir
from concourse import bass_utils
from concourse.masks import make_identity

D = 1024          # model dim
H = 16            # heads
DK = 64           # head dim
NCORES = 8
HPC = H // NCORES  # heads per core = 2
HD = HPC * DK      # local head dims = 128

F32 = mybir.dt.float32
F32R = mybir.dt.float32r
EXP = mybir.ActivationFunctionType.Exp


def build(S: int = 4096, no_collective: bool = False,
          phases: tuple = (1, 2, 3, 4)) -> bacc.Bacc:
    assert S % 512 == 0
    WIN = S // NCORES          # own output window rows
    NST = S // 512             # 512-wide chunks (proj s-tiles == q-windows)
    NSC = WIN // 128           # 128-row output s-chunks

    nc = bacc.Bacc("TRN2", target_bir_lowering=False, debug=False,
                   num_devices=NCORES)

    xT_d = nc.dram_tensor("xT", [S // 512, 128, 8, 512], F32R, kind="ExternalInput")
    wqT_d = nc.dram_tensor("wqT", [128, 8, HD], F32R, kind="ExternalInput")
    wkT_d = nc.dram_tensor("wkT", [128, 8, HD], F32R, kind="ExternalInput")
    wvT_d = nc.dram_tensor("wvT", [128, 8, HD], F32R, kind="ExternalInput")
    woT_d = nc.dram_tensor("woT", [128, 8, D], F32R, kind="ExternalInput")
    mask_d = nc.dram_tensor("mask", [128, 896], F32R, kind="ExternalInput")
    out_d = nc.dram_tensor("out", [WIN, D], F32, kind="ExternalOutput")

    rg = [list(range(NCORES))]
    run_attn = 2 in phases

    with tile.TileContext(nc) as tc:
        with (
            tc.tile_pool(name="persist", bufs=1) as persist,
            tc.tile_pool(name="dram", bufs=1, space="DRAM") as dram,
        ):
            # ---- persistent SBUF ----
            wq_sb = persist.tile([128, 8, HD], F32R)
            wk_sb = persist.tile([128, 8, HD], F32R)
            wv_sb = persist.tile([128, 8, HD], F32R)
            wo_sb = persist.tile([128, 8, D], F32R)
            mask_sb = persist.tile([128, 896], F32R)
            ident = persist.tile([128, 128], F32)
            ones64 = persist.tile([128, 64], F32R)
            # per-chunk projection outputs (fine-grained deps)
            qs = [persist.tile([128, 512], F32R, name=f"qs{i}")
                  for i in range(NST)]
            ks = [persist.tile([128, 512], F32R, name=f"ks{i}")
                  for i in range(NST)]
            va = [persist.tile([128, 4, 2 * (DK + 1)], F32R, name=f"va{i}")
                  for i in range(NST)]

            nc.sync.dma_start(wq_sb[:, 0:1, :], wqT_d.ap()[:, 0:1, :])
            nc.sync.dma_start(wq_sb[:, 1:, :], wqT_d.ap()[:, 1:, :])
            make_identity(nc, ident[:])
            ones_f32 = persist.tile([128, 64], F32)
            nc.vector.memset(ones_f32[:], 1.0)
            nc.vector.tensor_copy(ones64[:], ones_f32[:])
            for i in range(NST):
                for h in range(HPC):
                    nc.vector.tensor_copy(
                        va[i][:, :, h * (DK + 1) + DK:h * (DK + 1) + DK + 1],
                        ones_f32[:, 0:1].to_broadcast([128, 4, 1]))

            a2a_in = dram.tile([NCORES, HD, WIN], F32R)
            a2a_out = dram.tile([NCORES, HD, WIN], F32R)


            # ============ fused projection + attention pipeline ============
            with (
                tc.tile_pool(name="xs_pool", bufs=3) as xs_pool,
                tc.tile_pool(name="vt_pool", bufs=2) as vt_pool,
                tc.tile_pool(name="exp_pool", bufs=8) as exp_pool,
                tc.tile_pool(name="o_pool", bufs=4) as o_pool,
                tc.tile_pool(name="rc_pool", bufs=4) as rc_pool,
                tc.tile_pool(name="an_pool", bufs=4) as an_pool,
                tc.tile_pool(name="ps1", bufs=2, space="PSUM") as ps1,
                tc.tile_pool(name="ps_s", bufs=2, space="PSUM") as ps_s_pool,
                tc.tile_pool(name="ps_av", bufs=2, space="PSUM") as ps_av_pool,
            ):
                xs0 = xs_pool.tile([128, 8, 512], F32R, tag="xs", name="xs0")
                nc.sync.dma_start(xs0[:, 0:1, :], xT_d.ap()[0][:, 0:1, :])
                nc.sync.dma_start(xs0[:, 1:, :], xT_d.ap()[0][:, 1:, :])
                nc.sync.dma_start(wk_sb[:], wkT_d.ap())
                nc.sync.dma_start(wv_sb[:], wvT_d.ap())
                nc.sync.dma_start(mask_sb[:], mask_d.ap())
                for st in range(NST):
                    if st == NST // 2 and 4 in phases:
                        nc.sync.dma_start(wo_sb[:], woT_d.ap())
                    # ---- projections for chunk st ----
                    if 1 in phases:
                        if st == 0:
                            xs = xs0
                        else:
                            xs = xs_pool.tile([128, 8, 512], F32R, tag="xs", name="xs")
                            nc.sync.dma_start(xs[:], xT_d.ap()[st])
                        for w_sb, dst in ((wq_sb, qs[st]), (wk_sb, ks[st])):
                            ps = ps1.tile([128, 512], F32, tag="ps1", name="ps_qk")
                            for dt in range(8):
                                nc.tensor.matmul(ps[:], w_sb[:, dt, :], xs[:, dt, :],
                                                 start=(dt == 0), stop=(dt == 7))
                            nc.vector.tensor_copy(dst[:], ps[:])
                        psv = ps1.tile([128, 512], F32, tag="ps1", name="ps_v")
                        for dt in range(8):
                            nc.tensor.matmul(psv[:], wv_sb[:, dt, :], xs[:, dt, :],
                                             start=(dt == 0), stop=(dt == 7))
                        vT = vt_pool.tile([128, 512], F32)
                        nc.vector.tensor_copy(vT[:], psv[:])
                        for j in range(4):
                            pst = ps1.tile([128, 512], F32, tag="ps1", name="ps_t")
                            nc.tensor.transpose(pst[:, 0:128],
                                                vT[:, j * 128:(j + 1) * 128], ident[:])
                            nc.vector.tensor_copy(
                                va[st][:, j, :].rearrange(
                                    "p (h x) -> p h x", h=2)[:, :, 0:DK],
                                pst[:, 0:128].rearrange("p (h d) -> p h d", h=2))

                    # ---- attention for q-window t = st ----
                    if not run_attn:
                        continue
                    t = st
                    ngrp = 2 * (t + 1)
                    ps_av = [ps_av_pool.tile([DK + 1, 512], F32, tag="ps_av",
                                             name=f"ps_av{_h}")
                             for _h in range(HPC)]
                    for g in range(ngrp):
                        ps_grp = [ps_s_pool.tile([128, 1024], F32, tag="ps_s",
                                                 name=f"ps_sg{_h}")
                                  for _h in range(HPC)]
                        # h-interleaved score MMs: adjacent ops hit disjoint
                        # PE row-groups (base partition 0 / 64) and can run
                        # concurrently in the 32x32-tiled array on HW.
                        for ktl in range(2):
                            kt = 2 * g + ktl
                            kst, kidx = divmod(kt, 4)
                            for h in range(HPC):
                                hsl = slice(h * DK, (h + 1) * DK)
                                nc.tensor.matmul(
                                    ps_grp[h][:, ktl * 512:(ktl + 1) * 512],
                                    ks[kst][hsl, kidx * 128:(kidx + 1) * 128],
                                    qs[t][hsl, :], start=True, stop=True)
                        for h in range(HPC):
                            ex = exp_pool.tile([128, 1024], F32R)
                            nc.scalar.activation(ex[:], ps_grp[h][:], EXP,
                                                 scale=0.125)
                            for ktl in range(2):
                                kt = 2 * g + ktl
                                kst, kidx = divmod(kt, 4)
                                dj = kt - 4 * t
                                exs = ex[:, ktl * 512:(ktl + 1) * 512]
                                if 0 <= dj <= 3:
                                    nc.vector.tensor_mul(
                                        exs, exs,
                                        mask_sb[:, (3 - dj) * 128:(3 - dj) * 128 + 512])
                                nc.tensor.matmul(
                                    ps_av[h][:],
                                    va[kst][:, kidx,
                                            h * (DK + 1):(h + 1) * (DK + 1)],
                                    exs, start=(g == 0 and ktl == 0),
                                    stop=(g == ngrp - 1 and ktl == 1))
                    for h in range(HPC):
                        o_sb = o_pool.tile([DK + 1, 512], F32R)
                        nc.vector.tensor_copy(o_sb[:], ps_av[h][:])
                        ps_db = ps_av_pool.tile([DK + 1, 512], F32, tag="ps_av",
                                                name="ps_db")
                        nc.tensor.matmul(ps_db[0:DK, :], ones64[DK:DK + 1, :],
                                         o_sb[DK:DK + 1, :], start=True, stop=True)
                        rc = rc_pool.tile([DK, 512], F32)
                        nc.vector.reciprocal_approx_fast(rc[:], ps_db[0:DK, :])
                        an = an_pool.tile([DK, 512], F32R)
                        nc.vector.tensor_mul(an[:], o_sb[0:DK, :], rc[:])
                        for half in range(2):
                            w, off = divmod(t * 512 + half * 256, WIN)
                            nc.sync.dma_start(
                                a2a_in[w, h * DK:(h + 1) * DK, off:off + 256],
                                an[:, half * 256:(half + 1) * 256])

            # ================= Phase 3: AllToAll =================
            if 3 in phases:
                if no_collective:
                    nc.sync.dma_start(a2a_out[:], a2a_in[:])
                else:
                    nc.gpsimd.collective_compute(
                        "AllToAll", mybir.AluOpType.bypass, replica_groups=rg,
                        ins=[a2a_in.opt()], outs=[a2a_out.opt()])

            # ================= Phase 4: output projection =================
            if 4 in phases:
              with (
                tc.tile_pool(name="lhs_pool", bufs=8) as lhs_pool,
                tc.tile_pool(name="ob_pool", bufs=3) as ob_pool,
                tc.tile_pool(name="ps4", bufs=4, space="PSUM") as ps4,
              ):
                lhs_all = []
                for hb in range(8):
                    lt = lhs_pool.tile([128, WIN], F32R, tag="lhs",
                                       name=f"lhs{hb}")
                    nc.sync.dma_start(lt[:], a2a_out[hb])
                    lhs_all.append(lt)
                for c in range(NSC):
                    csl = slice(c * 128, (c + 1) * 128)
                    for do in range(2):
                        ps_o = ps4.tile([128, 512], F32)
                        for hb in range(8):
                            nc.tensor.matmul(ps_o[:], lhs_all[hb][:, csl],
                                             wo_sb[:, hb, do * 512:(do + 1) * 512],
                                             start=(hb == 0), stop=(hb == 7))
                        ob = ob_pool.tile([128, 512], F32)
                        nc.vector.tensor_copy(ob[:], ps_o[:])
                        nc.sync.dma_start(
                            out_d.ap()[csl, do * 512:(do + 1) * 512], ob[:])

    nc.compile()
    return nc


def make_mask() -> np.ndarray:
    """[Z Z Z T 1 1 1] -> [128, 896] multiplicative; slice (3-dj)*128 : +512
    masks a 128-k-tile at diag offset dj within a 512-q window."""
    m = np.zeros((128, 896), np.float32)
    k = np.arange(128)[:, None]
    q = np.arange(128)[None, :]
    m[:, 384:512] = (k <= q).astype(np.float32)
    m[:, 512:] = 1.0
    return m


def _arr_w(wT):
    # [D, M] -> [128, 8, M]: partition p holds din rows {dt*128+p}
    return np.ascontiguousarray(wT.reshape(8, 128, -1).transpose(1, 0, 2))


def make_in_maps(x, Wq, Wk, Wv, Wo, S):
    # xarr[st, p, dt, s] = x[512*st + s, 128*dt + p]
    xarr = np.ascontiguousarray(
        x.reshape(S // 512, 512, 8, 128).transpose(0, 3, 2, 1)).astype(np.float32)
    woT = _arr_w(Wo.T).astype(np.float32)
    mask = make_mask()
    in_maps = []
    for c in range(NCORES):
        hsl = slice(c * HD, (c + 1) * HD)
        in_maps.append({
            "xT": xarr,
            "wqT": _arr_w(Wq[hsl, :].T).astype(np.float32),
            "wkT": _arr_w(Wk[hsl, :].T).astype(np.float32),
            "wvT": _arr_w(Wv[hsl, :].T).astype(np.float32),
            "woT": woT,
            "mask": mask,
        })
    return in_maps


_cache: dict = {}


def kernel(x: np.ndarray, Wq: np.ndarray, Wk: np.ndarray, Wv: np.ndarray,
           Wo: np.ndarray) -> np.ndarray:
    B, S, Dm = x.shape
    assert B == 1 and Dm == D
    if S not in _cache:
        _cache[S] = build(S)
    nc = _cache[S]
    in_maps = make_in_maps(x, Wq, Wk, Wv, Wo, S)
    res = bass_utils.run_bass_kernel_spmd(nc, in_maps, core_ids=list(range(NCORES)))
    out = np.concatenate([res.results[c]["out"] for c in range(NCORES)], axis=0)
    return out.reshape(1, S, D).astype(np.float32)
